# revision 32
# baseline (speedup 1.0000x reference)
"""Trainium2 Bass kernel for banded multi-head attention (nn_MultiHeadAttention).

Full inputs in, full outputs out. Sharding: data-parallel over batch (8 cores,
one batch element each). Inside each core:
  q = (Wq/8)^T-proj(x)+bq/8; k = Wk-proj(c)+bk  (bf16 operands; FWL weight loads)
  v^T = c^T Wv^T (fp16; bias bv folded into bo' = bo + Wo bv on host)
  per (head, i-chunk of 256): S^T[j,i] = k_h^T q_h (banded j-tiles, bf16)
  E = exp(S^T) (ScalarE, fp16 out); E *= w  where w[j,i] = mask*band/(1+|i-j|)
  [numer; denom] = [V_h^T | 1]^T E  (ones-augmented PV matmul, fp32 PSUM,
  band-corner tiles restricted to their valid i-half)
  normalize: numer+denom cast-copied to SBUF fp16 (frees PSUM early), both
  heads' denom rows PE-broadcast col-tiled into po[1]'s upper partitions,
  one ScalarE copy to SBUF f32, reciprocal_approx_fast at base partition 0
  (the custom DVE op misreads PSUM operands and base_partition != 0 on HW),
  fp16 SBUF multiplies.  final = Wo-proj(O) + bo' with chunklets interleaved
  into attention chunks 2-3 as PE filler.
"""
import numpy as np

B, CH, T = 8, 512, 1024
H, KC, BLOCK = 8, 64, 256
P = 128
CB = CH // P       # 4 channel blocks
TTN = T // P       # 8 t-tiles
CHUNK = 256
NCH = T // CHUNK   # 4 chunks
VW = 66            # per-head V row width: 64 data + 1 ones + 1 pad

_CACHE = {}


def _chunk_jts(ch):
    jt0 = max(0, 2 * ch - 2)
    jt1 = min(TTN, 2 * ch + 4)
    return jt0, jt1


def _build_nc():
    import concourse.bass as bass
    import concourse.mybir as mybir
    import concourse.tile as tile
    from concourse import bacc
    from concourse.dve_ops import RECIP_APPROX_FAST_CONSTS, RECIPROCAL_APPROX_FAST

    f32 = mybir.dt.float32
    f32r = mybir.dt.float32r
    f16 = mybir.dt.float16
    bf16 = mybir.dt.bfloat16
    AF = mybir.ActivationFunctionType

    nc = bacc.Bacc("TRN2", target_bir_lowering=False, debug=False)
    x_d = nc.dram_tensor("x", [CH, T], bf16, kind="ExternalInput")
    c_d = nc.dram_tensor("c", [CH, T], bf16, kind="ExternalInput")
    wq_d = nc.dram_tensor("wqt", [CH, CH], bf16, kind="ExternalInput")
    wk_d = nc.dram_tensor("wkt", [CH, CH], bf16, kind="ExternalInput")
    wv_d = nc.dram_tensor("wvt", [CH, CH], bf16, kind="ExternalInput")
    wo_d = nc.dram_tensor("wot", [CH, CH], bf16, kind="ExternalInput")
    bqko_d = nc.dram_tensor("bqko", [P, 3 * CB], f32, kind="ExternalInput")
    w_d = nc.dram_tensor("w", [P, TTN, T], f16, kind="ExternalInput")
    ones_d = nc.dram_tensor("ones", [P, P], f16, kind="ExternalInput")
    out_d = nc.dram_tensor("out", [CH, T], f32, kind="ExternalOutput")

    def cbt(dram):  # [CH, X] dram -> [P, CB, X] load view
        return dram.rearrange("(cb p) t -> p cb t", p=P)

    with tile.TileContext(nc) as tc:
        with (
            tc.tile_pool(name="const", bufs=1) as const,
            tc.tile_pool(name="work", bufs=4) as work,
            tc.tile_pool(name="epool", bufs=4) as epool,
            tc.tile_pool(name="psS", bufs=2, space="PSUM") as psS,
            tc.tile_pool(name="psA", bufs=2, space="PSUM") as psA,
        ):
            # ---------- load constants & inputs (one DMA per tensor) ----------
            x_sb = const.tile([P, CB, T], bf16)
            c_sb = const.tile([P, CB, T], bf16)
            wq_sb = const.tile([P, CB, CH], bf16)
            wk_sb = const.tile([P, CB, CH], bf16)
            wv_sb = const.tile([P, CB, CH], bf16)
            wo_sb = const.tile([P, CB, CH], bf16)
            bqko_sb = const.tile([P, 3 * CB], f32)
            w_sb = const.tile([P, TTN, T], f16)
            ones_sb = const.tile([P, P], f16)
            # Load order/granularity chosen so compute can start early: first
            # x-half + Wq, then c + Wk/Wv, the band-weight matrix, Wo last.
            # Spread across issue queues to parallelize DGE dispatch.
            qs = [nc.sync, nc.gpsimd, nc.scalar]
            dmas = []
            dmas.append((x_sb[:, :, 0:512], cbt(x_d)[:, :, 0:512]))
            dmas.append((wq_sb, cbt(wq_d)))
            dmas.append((bqko_sb, bqko_d[:, :]))
            dmas.append((ones_sb, ones_d[:, :]))
            dmas.append((x_sb[:, :, 512:T], cbt(x_d)[:, :, 512:T]))
            dmas.append((c_sb[:, :, 0:512], cbt(c_d)[:, :, 0:512]))
            dmas.append((wk_sb, cbt(wk_d)))
            dmas.append((c_sb[:, :, 512:T], cbt(c_d)[:, :, 512:T]))
            dmas.append((wv_sb, cbt(wv_d)))
            dmas.append((w_sb[:, 0:4, :], w_d[:, 0:4, :]))
            dmas.append((w_sb[:, 4:TTN, :], w_d[:, 4:TTN, :]))
            dmas.append((wo_sb, cbt(wo_d)))
            for i, (dst, src) in enumerate(dmas):
                qs[i % len(qs)].dma_start(out=dst, in_=src)
            bq_sb = bqko_sb[:, 0:CB]
            bk_sb = bqko_sb[:, CB:2 * CB]
            bo_sb = bqko_sb[:, 2 * CB:3 * CB]

            q_sb = const.tile([P, CB, T], bf16)
            k_sb = const.tile([P, CB, T], bf16)
            v_sb = const.tile([P, TTN, H, VW], f16)
            o_sb = const.tile([P, CB, T], bf16)
            # ones column for the denominator row of the PV matmul: set the
            # whole tile to 1.0 once; data columns are overwritten by the V^T
            # copies. On GpSimd — it is idle and this keeps DVE free.
            nc.gpsimd.memset(v_sb, 1.0)

            # ---------- Q, K projections ----------
            # t2 inner so each weight block is stationary for 2 consecutive
            # matmuls (halves effective LDWEIGHTS traffic when the compiler
            # elides the repeat load).
            for wsb, bsb, src, dst in (
                (wq_sb, bq_sb, x_sb, q_sb),
                (wk_sb, bk_sb, c_sb, k_sb),
            ):
                for ob in range(CB):
                    pqs = [
                        psA.tile([P, 512], f32, tag="acc", name="pq")
                        for _ in range(2)
                    ]
                    for cb in range(CB):
                        for t2 in range(2):
                            nc.tensor.matmul(
                                pqs[t2],
                                wsb[:, cb, ob * P:(ob + 1) * P],
                                src[:, cb, t2 * 512:(t2 + 1) * 512],
                                start=(cb == 0),
                                stop=(cb == CB - 1),
                            )
                    for t2 in range(2):
                        nc.vector.tensor_scalar_add(
                            dst[:, ob, t2 * 512:(t2 + 1) * 512],
                            pqs[t2],
                            bsb[:, ob:ob + 1],
                        )

            # ---------- V^T projection (no bias: bv folded into bo') ----------
            for tt in range(TTN):
                pv = psA.tile([P, 512], f32, tag="acc", name="pv")
                for cb in range(CB):
                    nc.tensor.matmul(
                        pv,
                        c_sb[:, cb, tt * P:(tt + 1) * P],
                        wv_sb[:, cb, :],
                        start=(cb == 0),
                        stop=(cb == CB - 1),
                    )
                nc.scalar.activation(
                    v_sb[:, tt, :, 0:64],
                    pv.rearrange("p (h d) -> p h d", h=H),
                    AF.Copy,
                )

            # ---------- attention + interleaved output projection ----------
            out_view = out_d.rearrange("(cb p) t -> p cb t", p=P)
            fin_tiles = {}

            fin_done = {0: 0, 1: 0}

            def outproj_chunklet(t2, idx):
                """One (ob, 256-wide) slice of the output projection — emitted
                between attention iterations so its matmuls fill PE gaps
                without monopolizing the shared PSUM slots."""
                ob, th = idx // 2, idx % 2
                if t2 not in fin_tiles:
                    fin_tiles[t2] = work.tile(
                        [P, CB, 512], f32, tag="fin", name="fin", bufs=2
                    )
                fin = fin_tiles[t2]
                csl = slice(t2 * 512 + th * 256, t2 * 512 + (th + 1) * 256)
                pf = psA.tile([P, 256], f32, tag="acc", name="pf")
                for cb in range(CB):
                    nc.tensor.matmul(
                        pf,
                        wo_sb[:, cb, ob * P:(ob + 1) * P],
                        o_sb[:, cb, csl],
                        start=(cb == 0),
                        stop=(cb == CB - 1),
                    )
                nc.vector.tensor_scalar_add(
                    fin[:, ob, th * 256:(th + 1) * 256], pf, bo_sb[:, ob:ob + 1]
                )
                fin_done[t2] += 1
                if fin_done[t2] == 8:
                    nc.sync.dma_start(
                        out=out_view[:, :, t2 * 512:(t2 + 1) * 512], in_=fin
                    )

            for ch in range(NCH):
                jt0, jt1 = _chunk_jts(ch)
                njt = jt1 - jt0
                isl = slice(ch * CHUNK, (ch + 1) * CHUNK)
                o64c = work.tile([64, CB, CHUNK], bf16, tag="o64c", name="o64c", bufs=2)
                for m in range(H // 2):
                    # head pair (2m, 2m+1): score matmuls interleaved so the PE
                    # runs the K=64 row groups (0..63 / 64..127) concurrently
                    ps_pair = []
                    for par in (0, 1):
                        ps_p = psS.tile([P, 6, CHUNK], f32, tag="s", name="ps_p")
                        ps_pair.append(ps_p)
                    for u in range(njt):
                        jt = jt0 + u
                        for par in (0, 1):
                            hp = par * 64
                            nc.tensor.matmul(
                                ps_pair[par][:, u, :],
                                k_sb[hp:hp + KC, m, jt * P:(jt + 1) * P],
                                q_sb[hp:hp + KC, m, isl],
                                start=True,
                                stop=True,
                            )
                    # band-weight multiply: par 0 on DVE, par 1 on the idle
                    # GpSimd — halves the biggest DVE block; the extra GpSimd
                    # latency overlaps par 0's PV matmuls.
                    e_pair = []
                    for par in (0, 1):
                        e_t = epool.tile([P, 6, CHUNK], f16, name="e_t")
                        nc.scalar.activation(
                            e_t[:, 0:njt, :], ps_pair[par][:, 0:njt, :], AF.Exp
                        )
                        eng = nc.vector if par == 0 else nc.gpsimd
                        eng.tensor_mul(
                            e_t[:, 0:njt, :], e_t[:, 0:njt, :], w_sb[:, jt0:jt1, isl]
                        )
                        e_pair.append(e_t)
                    # PV then normalization. One DVE cast-copy moves numer AND
                    # denom [65,256] PSUM->SBUF f16 per head, freeing the po
                    # slot early (keeps the PV pipeline moving). The denom row
                    # is PE-broadcast (fp16 rank-1, into the pair's score PSUM
                    # which is free after exp), copied once to SBUF f32 by
                    # ScalarE, fast-approx-reciprocaled at base 0 (the custom
                    # op breaks at base partition != 0), and multiplied
                    # SBUF-x-SBUF at fp16 2x rate.
                    nb = work.tile([65, 2, CHUNK], f16, tag="nb", name="nb")
                    db = work.tile([64, 2, CHUNK], f32, tag="db", name="db")
                    rdb = work.tile([64, 2, CHUNK], f16, tag="rdb", name="rdb")
                    rc = RECIP_APPROX_FAST_CONSTS
                    po_pair = []
                    for par in (0, 1):
                        h = 2 * m + par
                        po = psA.tile([P, 512], f32, tag="acc", name="po")
                        po_pair.append(po)
                        # full j-tiles first (the first matmul must cover the
                        # whole accumulation region), then band-corner tiles
                        # restricted to their valid i-half (E*w = 0 outside).
                        full, corners = [], []
                        for u in range(njt):
                            jt = jt0 + u
                            nu = 2 * ch - jt
                            if nu == 2:
                                corners.append((u, slice(0, 128)))
                            elif nu == -3:
                                corners.append((u, slice(128, 256)))
                            else:
                                full.append(u)
                        order = [(u, slice(0, CHUNK)) for u in full] + corners
                        for idx, (u, csl) in enumerate(order):
                            jt = jt0 + u
                            nc.tensor.matmul(
                                po[0:65, csl],
                                v_sb[:, jt, h, 0:65],
                                e_pair[par][:, u, csl],
                                start=(idx == 0),
                                stop=(idx == len(order) - 1),
                            )
                        nc.vector.tensor_copy(
                            nb[:, par, :], po[0:65, 0:CHUNK]
                        )
                    # broadcast both heads' recip rows at once into po[1]'s
                    # unused upper partitions (col-tiled output at base 64) —
                    # NOT the score PSUM, so the next pair's score matmuls
                    # aren't gated on this pair's normalization chain (keeps
                    # PE dense -> HAM stays at full clock).
                    pbc = po_pair[1][64:128, :].rearrange(
                        "p (a i) -> p a i", a=2
                    )
                    nc.tensor.matmul(
                        pbc,
                        ones_sb[64:65, 0:64],
                        nb[64:65, :, :],
                        start=True,
                        stop=True,
                        tile_position=(64, 64),
                    )
                    nc.scalar.activation(db, pbc, AF.Copy)
                    nc.vector._custom_dve(
                        RECIPROCAL_APPROX_FAST,
                        out=rdb,
                        in0=db,
                        s0=rc["s0"],
                        s1=rc["s1"],
                        imm2=rc["imm2"],
                    )
                    for par in (0, 1):
                        if par == 0:
                            nc.vector.tensor_mul(
                                o_sb[0:64, m, isl],
                                nb[0:64, par, :],
                                rdb[:, par, :],
                            )
                        else:
                            nc.vector.tensor_mul(
                                o64c[:, m, :],
                                nb[0:64, par, :],
                                rdb[:, par, :],
                            )
                        # interleave output-projection chunklets: t2=0 into
                        # ch=2, and t2=1's th=0 half (reads o[512:768], ready
                        # after ch=2) into ch=3 — PE filler between pairs.
                        if ch == 2:
                            outproj_chunklet(0, 2 * m + par)
                        elif ch == 3 and par == 1:
                            outproj_chunklet(1, 2 * m)
                nc.sync.dma_start(out=o_sb[64:128, :, isl], in_=o64c)

            for idx in (1, 3, 5, 7):
                outproj_chunklet(1, idx)

    nc.compile()
    return nc


def _host_prep(attn_mask, Wq, bq, Wk, bk, Wv, bv, Wo, bo):
    """Precompute per-core shared inputs (weights layouts + combined band/bias/mask)."""
    import ml_dtypes

    bfloat16 = ml_dtypes.bfloat16
    scale = 1.0 / np.sqrt(KC)
    wqt = np.ascontiguousarray((np.asarray(Wq) * scale).T.astype(bfloat16))
    wkt = np.ascontiguousarray(np.asarray(Wk).T.astype(bfloat16))
    wvt = np.ascontiguousarray(np.asarray(Wv).T.astype(bfloat16))
    wot = np.ascontiguousarray(np.asarray(Wo).T.astype(bfloat16))
    # bv passes through attention unchanged (softmax weights sum to 1), so it
    # folds into the output projection: bo' = bo + Wo @ bv.
    bo_eff = (np.asarray(bo) + np.asarray(Wo) @ np.asarray(bv)).astype(np.float32)
    bqko = np.concatenate(
        [
            (np.asarray(bq) * scale).astype(np.float32).reshape(CB, P).T,
            np.asarray(bk).astype(np.float32).reshape(CB, P).T,
            bo_eff.reshape(CB, P).T,
        ],
        axis=1,
    )
    bqko = np.ascontiguousarray(bqko)

    r = np.arange(T)
    diff = np.abs(r[None, :] - r[:, None])            # |i - j|
    w_mat = 1.0 / (1.0 + diff.astype(np.float64))      # exp(-log1p|i-j|)
    band = diff <= BLOCK
    mask = np.asarray(attn_mask).reshape(T, T) != 0    # [i, j]
    w_eff = np.where(band & mask, w_mat, 0.0)          # [i, j]
    w_T = w_eff.T                                      # [j, i]
    w_planes = np.ascontiguousarray(
        w_T.reshape(TTN, P, T).transpose(1, 0, 2).astype(np.float16)
    )
    return dict(
        wqt=wqt, wkt=wkt, wvt=wvt, wot=wot,
        bqko=bqko, w=w_planes,
        ones=np.ones((P, P), dtype=np.float16),
    )


def kernel(x, c, attn_mask, Wq, bq, Wk, bk, Wv, bv, Wo, bo, _trace=False):
    import ml_dtypes
    from concourse.bass_utils import run_bass_kernel_spmd

    if "nc" not in _CACHE:
        _CACHE["nc"] = _build_nc()
    nc = _CACHE["nc"]

    shared = _host_prep(attn_mask, Wq, bq, Wk, bk, Wv, bv, Wo, bo)
    bfloat16 = ml_dtypes.bfloat16
    x = np.ascontiguousarray(np.asarray(x).astype(bfloat16))
    c = np.ascontiguousarray(np.asarray(c).astype(bfloat16))
    in_maps = [dict(shared, x=x[b], c=c[b]) for b in range(B)]
    kwargs = {}
    if _trace:
        kwargs = dict(trace=True)
    res = run_bass_kernel_spmd(nc, in_maps, core_ids=list(range(B)), **kwargs)
    out = np.stack([res.results[b]["out"] for b in range(B)], axis=0)
    if _trace:
        _CACHE["last_results"] = res
    return out


# revision 34
# speedup vs baseline: 1.0489x; 1.0489x over previous
"""Trainium2 Bass kernel for banded multi-head attention (nn_MultiHeadAttention).

Full inputs in, full outputs out. Sharding: data-parallel over batch (8 cores,
one batch element each). Inside each core:
  q = (Wq/8)^T-proj(x)+bq/8; k = Wk-proj(c)+bk  (bf16 operands; FWL weight loads)
  v^T = c^T Wv^T (fp16; bias bv folded into bo' = bo + Wo bv on host)
  per (head, i-chunk of 256): S^T[j,i] = k_h^T q_h (banded j-tiles, bf16)
  E = exp(S^T) (ScalarE, fp16 out); E *= w  where w[j,i] = mask*band/(1+|i-j|)
  [numer; denom] = [V_h^T | 1]^T E  (ones-augmented PV matmul, fp32 PSUM,
  band-corner tiles restricted to their valid i-half)
  normalize: numer+denom cast-copied to SBUF fp16 (frees PSUM early), both
  heads' denom rows PE-broadcast col-tiled into po[1]'s upper partitions,
  one ScalarE copy to SBUF f32, reciprocal_approx_fast at base partition 0
  (the custom DVE op misreads PSUM operands and base_partition != 0 on HW),
  fp16 SBUF multiplies.  final = Wo-proj(O) + bo' with chunklets interleaved
  into attention chunks 2-3 as PE filler.
"""
import numpy as np

B, CH, T = 8, 512, 1024
H, KC, BLOCK = 8, 64, 256
P = 128
CB = CH // P       # 4 channel blocks
TTN = T // P       # 8 t-tiles
CHUNK = 256
NCH = T // CHUNK   # 4 chunks
VW = 66            # per-head V row width: 64 data + 1 ones + 1 pad

_CACHE = {}


def _chunk_jts(ch):
    jt0 = max(0, 2 * ch - 2)
    jt1 = min(TTN, 2 * ch + 4)
    return jt0, jt1


def _build_nc():
    import concourse.bass as bass
    import concourse.mybir as mybir
    import concourse.tile as tile
    from concourse import bacc
    from concourse.dve_ops import RECIP_APPROX_FAST_CONSTS, RECIPROCAL_APPROX_FAST

    f32 = mybir.dt.float32
    f32r = mybir.dt.float32r
    f16 = mybir.dt.float16
    bf16 = mybir.dt.bfloat16
    AF = mybir.ActivationFunctionType

    nc = bacc.Bacc("TRN2", target_bir_lowering=False, debug=False)
    x_d = nc.dram_tensor("x", [CH, T], bf16, kind="ExternalInput")
    c_d = nc.dram_tensor("c", [CH, T], bf16, kind="ExternalInput")
    wq_d = nc.dram_tensor("wqt", [CH, CH], bf16, kind="ExternalInput")
    wk_d = nc.dram_tensor("wkt", [CH, CH], bf16, kind="ExternalInput")
    wv_d = nc.dram_tensor("wvt", [CH, CH], bf16, kind="ExternalInput")
    wo_d = nc.dram_tensor("wot", [CH, CH], bf16, kind="ExternalInput")
    bqko_d = nc.dram_tensor("bqko", [P, 3 * CB], f32, kind="ExternalInput")
    w_d = nc.dram_tensor("w", [P, TTN, T], f16, kind="ExternalInput")
    ones_d = nc.dram_tensor("ones", [P, P], f16, kind="ExternalInput")
    out_d = nc.dram_tensor("out", [CH, T], f32, kind="ExternalOutput")

    def cbt(dram):  # [CH, X] dram -> [P, CB, X] load view
        return dram.rearrange("(cb p) t -> p cb t", p=P)

    with tile.TileContext(nc) as tc:
        with (
            tc.tile_pool(name="const", bufs=1) as const,
            tc.tile_pool(name="work", bufs=4) as work,
            tc.tile_pool(name="epool", bufs=4) as epool,
            tc.tile_pool(name="psS", bufs=2, space="PSUM") as psS,
            tc.tile_pool(name="psA", bufs=2, space="PSUM") as psA,
        ):
            # ---------- load constants & inputs (one DMA per tensor) ----------
            x_sb = const.tile([P, CB, T], bf16)
            c_sb = const.tile([P, CB, T], bf16)
            wq_sb = const.tile([P, CB, CH], bf16)
            wk_sb = const.tile([P, CB, CH], bf16)
            wv_sb = const.tile([P, CB, CH], bf16)
            wo_sb = const.tile([P, CB, CH], bf16)
            bqko_sb = const.tile([P, 3 * CB], f32)
            w_sb = const.tile([P, TTN, T], f16)
            ones_sb = const.tile([P, P], f16)
            # Load order/granularity chosen so compute can start early: first
            # x-half + Wq, then c + Wk/Wv, the band-weight matrix, Wo last.
            # Spread across issue queues to parallelize DGE dispatch.
            qs = [nc.sync, nc.gpsimd, nc.scalar]
            dmas = []
            dmas.append((x_sb[:, :, 0:512], cbt(x_d)[:, :, 0:512]))
            dmas.append((wq_sb, cbt(wq_d)))
            dmas.append((bqko_sb, bqko_d[:, :]))
            dmas.append((ones_sb, ones_d[:, :]))
            dmas.append((x_sb[:, :, 512:T], cbt(x_d)[:, :, 512:T]))
            dmas.append((c_sb[:, :, 0:512], cbt(c_d)[:, :, 0:512]))
            dmas.append((wk_sb, cbt(wk_d)))
            dmas.append((c_sb[:, :, 512:T], cbt(c_d)[:, :, 512:T]))
            dmas.append((wv_sb, cbt(wv_d)))
            dmas.append((w_sb[:, 0:4, :], w_d[:, 0:4, :]))
            dmas.append((w_sb[:, 4:TTN, :], w_d[:, 4:TTN, :]))
            dmas.append((wo_sb, cbt(wo_d)))
            for i, (dst, src) in enumerate(dmas):
                qs[i % len(qs)].dma_start(out=dst, in_=src)
            bq_sb = bqko_sb[:, 0:CB]
            bk_sb = bqko_sb[:, CB:2 * CB]
            bo_sb = bqko_sb[:, 2 * CB:3 * CB]

            q_sb = const.tile([P, CB, T], bf16)
            k_sb = const.tile([P, CB, T], bf16)
            v_sb = const.tile([P, TTN, H, VW], f16)
            o_sb = const.tile([P, CB, T], bf16)
            # ones column for the denominator row of the PV matmul: set the
            # whole tile to 1.0 once; data columns are overwritten by the V^T
            # copies. On GpSimd — it is idle and this keeps DVE free.
            nc.gpsimd.memset(v_sb, 1.0)

            # ---------- Q, K projections ----------
            # t2 inner so each weight block is stationary for 2 consecutive
            # matmuls (halves effective LDWEIGHTS traffic when the compiler
            # elides the repeat load).
            for wsb, bsb, src, dst in (
                (wq_sb, bq_sb, x_sb, q_sb),
                (wk_sb, bk_sb, c_sb, k_sb),
            ):
                for ob in range(CB):
                    pqs = [
                        psA.tile([P, 512], f32, tag="acc", name="pq")
                        for _ in range(2)
                    ]
                    for cb in range(CB):
                        for t2 in range(2):
                            nc.tensor.matmul(
                                pqs[t2],
                                wsb[:, cb, ob * P:(ob + 1) * P],
                                src[:, cb, t2 * 512:(t2 + 1) * 512],
                                start=(cb == 0),
                                stop=(cb == CB - 1),
                            )
                    for t2 in range(2):
                        nc.vector.tensor_scalar_add(
                            dst[:, ob, t2 * 512:(t2 + 1) * 512],
                            pqs[t2],
                            bsb[:, ob:ob + 1],
                        )

            # ---------- V^T projection (no bias: bv folded into bo') ----------
            for tt in range(TTN):
                pv = psA.tile([P, 512], f32, tag="acc", name="pv")
                for cb in range(CB):
                    nc.tensor.matmul(
                        pv,
                        c_sb[:, cb, tt * P:(tt + 1) * P],
                        wv_sb[:, cb, :],
                        start=(cb == 0),
                        stop=(cb == CB - 1),
                    )
                nc.scalar.activation(
                    v_sb[:, tt, :, 0:64],
                    pv.rearrange("p (h d) -> p h d", h=H),
                    AF.Copy,
                )

            # ---------- attention + interleaved output projection ----------
            out_view = out_d.rearrange("(cb p) t -> p cb t", p=P)
            fin_tiles = {}

            fin_done = {0: 0, 1: 0}

            def outproj_chunklet(t2, idx):
                """One (ob, 256-wide) slice of the output projection — emitted
                between attention iterations so its matmuls fill PE gaps
                without monopolizing the shared PSUM slots."""
                ob, th = idx // 2, idx % 2
                if t2 not in fin_tiles:
                    fin_tiles[t2] = work.tile(
                        [P, CB, 512], f32, tag="fin", name="fin", bufs=2
                    )
                fin = fin_tiles[t2]
                csl = slice(t2 * 512 + th * 256, t2 * 512 + (th + 1) * 256)
                pf = psA.tile([P, 256], f32, tag="acc", name="pf")
                for cb in range(CB):
                    nc.tensor.matmul(
                        pf,
                        wo_sb[:, cb, ob * P:(ob + 1) * P],
                        o_sb[:, cb, csl],
                        start=(cb == 0),
                        stop=(cb == CB - 1),
                    )
                nc.vector.tensor_scalar_add(
                    fin[:, ob, th * 256:(th + 1) * 256], pf, bo_sb[:, ob:ob + 1]
                )
                fin_done[t2] += 1
                if fin_done[t2] == 8:
                    nc.sync.dma_start(
                        out=out_view[:, :, t2 * 512:(t2 + 1) * 512], in_=fin
                    )

            for ch in range(NCH):
                jt0, jt1 = _chunk_jts(ch)
                njt = jt1 - jt0
                isl = slice(ch * CHUNK, (ch + 1) * CHUNK)
                o64c = work.tile([64, CB, CHUNK], bf16, tag="o64c", name="o64c", bufs=2)
                for m in range(H // 2):
                    # head pair (2m, 2m+1): score matmuls interleaved so the PE
                    # runs the K=64 row groups (0..63 / 64..127) concurrently
                    ps_pair = []
                    for par in (0, 1):
                        ps_p = psS.tile([P, 6, CHUNK], f32, tag="s", name="ps_p")
                        ps_pair.append(ps_p)
                    for u in range(njt):
                        jt = jt0 + u
                        for par in (0, 1):
                            hp = par * 64
                            nc.tensor.matmul(
                                ps_pair[par][:, u, :],
                                k_sb[hp:hp + KC, m, jt * P:(jt + 1) * P],
                                q_sb[hp:hp + KC, m, isl],
                                start=True,
                                stop=True,
                            )
                    e_pair = []
                    for par in (0, 1):
                        e_t = epool.tile([P, 6, CHUNK], f16, name="e_t")
                        nc.scalar.activation(
                            e_t[:, 0:njt, :], ps_pair[par][:, 0:njt, :], AF.Exp
                        )
                        nc.vector.tensor_mul(
                            e_t[:, 0:njt, :], e_t[:, 0:njt, :], w_sb[:, jt0:jt1, isl]
                        )
                        e_pair.append(e_t)
                    # PV then normalization. One DVE cast-copy moves numer AND
                    # denom [65,256] PSUM->SBUF f16 per head, freeing the po
                    # slot early (keeps the PV pipeline moving). The denom row
                    # is PE-broadcast (fp16 rank-1, into the pair's score PSUM
                    # which is free after exp), copied once to SBUF f32 by
                    # ScalarE, fast-approx-reciprocaled at base 0 (the custom
                    # op breaks at base partition != 0), and multiplied
                    # SBUF-x-SBUF at fp16 2x rate.
                    nb = work.tile([65, 2, CHUNK], f16, tag="nb", name="nb")
                    db = work.tile([64, 2, CHUNK], f32, tag="db", name="db")
                    rdb = work.tile([64, 2, CHUNK], f16, tag="rdb", name="rdb")
                    rc = RECIP_APPROX_FAST_CONSTS
                    po_pair = []
                    for par in (0, 1):
                        h = 2 * m + par
                        po = psA.tile([P, 512], f32, tag="acc", name="po")
                        po_pair.append(po)
                        # full j-tiles first (the first matmul must cover the
                        # whole accumulation region), then band-corner tiles
                        # restricted to their valid i-half (E*w = 0 outside).
                        full, corners = [], []
                        for u in range(njt):
                            jt = jt0 + u
                            nu = 2 * ch - jt
                            if nu == 2:
                                corners.append((u, slice(0, 128)))
                            elif nu == -3:
                                corners.append((u, slice(128, 256)))
                            else:
                                full.append(u)
                        order = [(u, slice(0, CHUNK)) for u in full] + corners
                        for idx, (u, csl) in enumerate(order):
                            jt = jt0 + u
                            nc.tensor.matmul(
                                po[0:65, csl],
                                v_sb[:, jt, h, 0:65],
                                e_pair[par][:, u, csl],
                                start=(idx == 0),
                                stop=(idx == len(order) - 1),
                            )
                        # split the numer+denom cast-copies across engines:
                        # par 0 on DVE, par 1 on ScalarE (rebalances the two
                        # mid engines' per-pair chain latency).
                        if par == 0:
                            nc.vector.tensor_copy(
                                nb[:, par, :], po[0:65, 0:CHUNK]
                            )
                        else:
                            nc.scalar.activation(
                                nb[:, par, :], po[0:65, 0:CHUNK], AF.Copy
                            )
                    # broadcast both heads' recip rows at once into po[1]'s
                    # unused upper partitions (col-tiled output at base 64) —
                    # NOT the score PSUM, so the next pair's score matmuls
                    # aren't gated on this pair's normalization chain (keeps
                    # PE dense -> HAM stays at full clock).
                    pbc = po_pair[1][64:128, :].rearrange(
                        "p (a i) -> p a i", a=2
                    )
                    nc.tensor.matmul(
                        pbc,
                        ones_sb[64:65, 0:64],
                        nb[64:65, :, :],
                        start=True,
                        stop=True,
                        tile_position=(64, 64),
                    )
                    nc.scalar.activation(db, pbc, AF.Copy)
                    nc.vector._custom_dve(
                        RECIPROCAL_APPROX_FAST,
                        out=rdb,
                        in0=db,
                        s0=rc["s0"],
                        s1=rc["s1"],
                        imm2=rc["imm2"],
                    )
                    for par in (0, 1):
                        if par == 0:
                            nc.vector.tensor_mul(
                                o_sb[0:64, m, isl],
                                nb[0:64, par, :],
                                rdb[:, par, :],
                            )
                        else:
                            nc.vector.tensor_mul(
                                o64c[:, m, :],
                                nb[0:64, par, :],
                                rdb[:, par, :],
                            )
                        # interleave output-projection chunklets: t2=0 into
                        # ch=2, and t2=1's th=0 half (reads o[512:768], ready
                        # after ch=2) into ch=3 — PE filler between pairs.
                        if ch == 2:
                            outproj_chunklet(0, 2 * m + par)
                        elif ch == 3 and par == 1:
                            outproj_chunklet(1, 2 * m)
                nc.sync.dma_start(out=o_sb[64:128, :, isl], in_=o64c)

            for idx in (1, 3, 5, 7):
                outproj_chunklet(1, idx)

    nc.compile()
    return nc


def _host_prep(attn_mask, Wq, bq, Wk, bk, Wv, bv, Wo, bo):
    """Precompute per-core shared inputs (weights layouts + combined band/bias/mask)."""
    import ml_dtypes

    bfloat16 = ml_dtypes.bfloat16
    scale = 1.0 / np.sqrt(KC)
    wqt = np.ascontiguousarray((np.asarray(Wq) * scale).T.astype(bfloat16))
    wkt = np.ascontiguousarray(np.asarray(Wk).T.astype(bfloat16))
    wvt = np.ascontiguousarray(np.asarray(Wv).T.astype(bfloat16))
    wot = np.ascontiguousarray(np.asarray(Wo).T.astype(bfloat16))
    # bv passes through attention unchanged (softmax weights sum to 1), so it
    # folds into the output projection: bo' = bo + Wo @ bv.
    bo_eff = (np.asarray(bo) + np.asarray(Wo) @ np.asarray(bv)).astype(np.float32)
    bqko = np.concatenate(
        [
            (np.asarray(bq) * scale).astype(np.float32).reshape(CB, P).T,
            np.asarray(bk).astype(np.float32).reshape(CB, P).T,
            bo_eff.reshape(CB, P).T,
        ],
        axis=1,
    )
    bqko = np.ascontiguousarray(bqko)

    r = np.arange(T)
    diff = np.abs(r[None, :] - r[:, None])            # |i - j|
    w_mat = 1.0 / (1.0 + diff.astype(np.float64))      # exp(-log1p|i-j|)
    band = diff <= BLOCK
    mask = np.asarray(attn_mask).reshape(T, T) != 0    # [i, j]
    w_eff = np.where(band & mask, w_mat, 0.0)          # [i, j]
    w_T = w_eff.T                                      # [j, i]
    w_planes = np.ascontiguousarray(
        w_T.reshape(TTN, P, T).transpose(1, 0, 2).astype(np.float16)
    )
    return dict(
        wqt=wqt, wkt=wkt, wvt=wvt, wot=wot,
        bqko=bqko, w=w_planes,
        ones=np.ones((P, P), dtype=np.float16),
    )


def kernel(x, c, attn_mask, Wq, bq, Wk, bk, Wv, bv, Wo, bo, _trace=False):
    import ml_dtypes
    from concourse.bass_utils import run_bass_kernel_spmd

    if "nc" not in _CACHE:
        _CACHE["nc"] = _build_nc()
    nc = _CACHE["nc"]

    shared = _host_prep(attn_mask, Wq, bq, Wk, bk, Wv, bv, Wo, bo)
    bfloat16 = ml_dtypes.bfloat16
    x = np.ascontiguousarray(np.asarray(x).astype(bfloat16))
    c = np.ascontiguousarray(np.asarray(c).astype(bfloat16))
    in_maps = [dict(shared, x=x[b], c=c[b]) for b in range(B)]
    kwargs = {}
    if _trace:
        kwargs = dict(trace=True)
    res = run_bass_kernel_spmd(nc, in_maps, core_ids=list(range(B)), **kwargs)
    out = np.stack([res.results[b]["out"] for b in range(B)], axis=0)
    if _trace:
        _CACHE["last_results"] = res
    return out
